# revision 7
# baseline (speedup 1.0000x reference)
"""Multi-head attention (B=2, N=4096, C=512, H=8) on 8 TRN2 NeuronCores.

Sharding: core c handles batch b = c//4 and heads {2*(c%4), 2*(c%4)+1}
(data parallel over B, tensor parallel over heads).  Each core computes its
two heads' full attention plus their slice of the output projection; the
per-core [C, N] projection partials are summed per batch on the host
(the "proj all-reduce") and the projection bias is added there too (with
the attention v-bias folded in: out = Wp(O0 + vb) + pb = Wp O0 + (Wp vb + pb)).

Device-side math per core:
  qT/kT   = Wq/Wk_blk @ x^T + b           [128 (2h x 64d), 4096] fp32 PSUM
  q8/k8   = fp8e4(qT/kT)                  repacked via DRAM roundtrip into
                                          [32 (d0), 2h, 2g, 2t, 8ic, 512] so the
                                          score matmul runs fp8 DoubleRow with
                                          the 64-d contraction split into 2
                                          groups of 32 on partitions 0:32.
  v_nat   = x_jt^T @ Wv  (vb folded out)  [128 (j), 2h x 64d] -> vno fp8
                                          [128, 32jt, (64|1|64|1)] ones cols
  S^T     = DR-matmul(k8, q8)             [128 (j), 2h, 512i] fp32 PSUM
  P^T     = exp(SCALE * S^T) -> fp8       ACT exp (most units) or DVE
                                          bf16-stage + GPSIMD pow (rest)
  O_raw^T = DR-matmul over jt PAIRS:      lhsT = vno[:, 2t:2t+2, h slice],
            rhs = p2[:, 2g, h, :]         -> acc[h] [65, 512] (row 64 = denom)
  O^T     = O_raw^T * (1/denom)           DVE recip + Pool bcast + DVE mult
  out^T  += Wp_blk^T @ O^T                [512, 4096] fp32 -> DRAM
"""

import os
import numpy as np
import ml_dtypes

SC_BUFS = int(os.environ.get("SC_BUFS", "3"))
P2_BUFS = int(os.environ.get("P2_BUFS", "18"))
STG_BUFS = int(os.environ.get("STG_BUFS", "4"))
PDEFER = int(os.environ.get("PDEFER", "2"))
POOL_NUM = int(os.environ.get("POOL_NUM", "7"))   # pool pairs per POOL_DEN
POOL_DEN = int(os.environ.get("POOL_DEN", "10"))
POOL_NUM0 = int(os.environ.get("POOL_NUM0", "1"))  # ...during i-chunk 0
POOL_DEN0 = int(os.environ.get("POOL_DEN0", "2"))
AB_DIRECT = int(os.environ.get("AB_DIRECT", "1"))
WARM = int(os.environ.get("WARM", "24"))
FLUSH0 = int(os.environ.get("FLUSH0", "27"))  # unit where ic-0 attnV flush starts

B, N, C = 2, 4096, 512
H, Dh = 8, 64
SCALE = Dh**-0.5
NCORES = 8
HPC = 2  # heads per core
ICW = 512  # i-chunk width
NIC = N // ICW  # 8
JTW = 128  # j-tile width
NJT = N // JTW  # 32
NPAIR = NJT // 2  # 16 jt pairs per i-chunk

_BF16 = ml_dtypes.bfloat16
_F8 = ml_dtypes.float8_e4m3

_cached_nc = {}


def _build_nc(reps=1):
    import concourse.bacc as bacc
    import concourse.tile as tile
    import concourse.mybir as mybir

    f32 = mybir.dt.float32
    bf16 = mybir.dt.bfloat16
    fp8 = mybir.dt.float8e4
    Exp = mybir.ActivationFunctionType.Exp
    mult = mybir.AluOpType.mult
    pow_op = mybir.AluOpType.pow
    DR = mybir.MatmulPerfMode.DoubleRow

    nc = bacc.Bacc("TRN2", target_bir_lowering=False, debug=False)

    xt_d = nc.dram_tensor("xt", [C, N], bf16, kind="ExternalInput").ap()
    wqkv_d = nc.dram_tensor("wqkv", [C, 3 * 128], bf16, kind="ExternalInput").ap()
    wp_d = nc.dram_tensor("wp", [128, C], bf16, kind="ExternalInput").ap()
    bqkv_d = nc.dram_tensor("bqkv", [128, 2], f32, kind="ExternalInput").ap()
    out_d = nc.dram_tensor("out", [C, N], f32, kind="ExternalOutput").ap()
    # DRAM scratch for the q8/k8 partition repack: [p, t(q|k), ic, i]
    qk8_d = nc.dram_tensor("qk8s", [128, 2, NIC, ICW], fp8, kind="ExternalOutput").ap()

    def is_pool(idx):
        # Pair-aware: at most one Pool unit per jt-pair (so ACT+Pool overlap
        # within a pair), alternating which member, Bresenham-spread pairs.
        t, g = divmod(idx, 2)
        if idx < NJT:
            pair_pool = (t * POOL_NUM0) % POOL_DEN0 < POOL_NUM0
        else:
            pair_pool = (t * POOL_NUM) % POOL_DEN < POOL_NUM
        return pair_pool and g == (t % 2)

    with tile.TileContext(nc) as tc:
        with (
            tc.tile_pool(name="ps", space="PSUM", bufs=2) as ps,
            tc.tile_pool(name="sp", bufs=2) as sp,
            tc.tile_pool(name="pe", bufs=1) as pe,
        ):
            # --- persistent SBUF tensors ---
            xt = [
                pe.tile([128, N], bf16, tag=f"xt{i}", name=f"xt{i}") for i in range(4)
            ]

            def load_x():
                for i in range(NIC):
                    for ct in range(4):
                        nc.sync.dma_start(
                            out=xt[ct][:, i * ICW : (i + 1) * ICW],
                            in_=xt_d[
                                ct * 128 : (ct + 1) * 128, i * ICW : (i + 1) * ICW
                            ],
                        )

            wqkv = pe.tile([128, 4, 3 * 128], bf16, tag="wqkv", name="wqkv")
            nc.sync.dma_start(
                out=wqkv[:], in_=wqkv_d.rearrange("(ct p) m -> p ct m", p=128)
            )
            wp = pe.tile([128, C], bf16, tag="wp", name="wp")
            nc.sync.dma_start(out=wp[:], in_=wp_d[:, :])
            bqkv = pe.tile([128, 2], f32, tag="bqkv", name="bqkv")
            nc.sync.dma_start(out=bqkv[:], in_=bqkv_d[:, :])

            # q8/k8 in d-split layout: [32 (d0), h, g, t, ic, i]
            qk8 = pe.tile([32, HPC, 2, 2, NIC, ICW], fp8, tag="qk8", name="qk8")
            # v natural layout + ones cols: [j0, jt, (64 v | 1 | 64 v | 1)]
            vno = pe.tile([128, NJT, HPC * (Dh + 1)], fp8, tag="vno", name="vno")
            onorm = pe.tile([128, N], bf16, tag="onorm", name="onorm")
            base = pe.tile([128, HPC, ICW], f32, tag="base", name="base")
            nc.vector.memset(base[:], float(np.exp(SCALE)))
            nc.vector.memset(vno[:, :, Dh : Dh + 1], 1.0)
            nc.vector.memset(vno[:, :, 2 * Dh + 1 : 2 * Dh + 2], 1.0)

            # PE pre-warm (pstate ramp) while input DMAs land.
            warm = pe.tile([128, 128], bf16, tag="warm", name="warm")
            nc.vector.memset(warm[:], 0.0)
            wps = ps.tile([128, HPC, ICW], f32, tag="sc", bufs=SC_BUFS, name="wps")
            for _ in range(WARM):
                nc.tensor.matmul(
                    wps[:, 0, 0:128], lhsT=warm[:], rhs=warm[:], start=True, stop=True
                )

            # --- QKV production for one 512-column chunk, in parts ---
            qkv_state = {}

            def qkv_qk(i):
                isl = slice(i * ICW, (i + 1) * ICW)
                qk = ps.tile([128, 2, ICW], f32, tag="sc", bufs=SC_BUFS, name="qk")
                qkv_state[i] = qk
                for part in range(2):
                    for ct in range(4):
                        nc.tensor.matmul(
                            qk[:, part, :],
                            lhsT=wqkv[:, ct, part * 128 : (part + 1) * 128],
                            rhs=xt[ct][:, isl],
                            start=(ct == 0),
                            stop=(ct == 3),
                        )

            def qkv_repack(i):
                qk = qkv_state.pop(i)
                qf = sp.tile([128, 2, ICW], fp8, tag="qf", bufs=2, name="qf")
                nc.vector.tensor_scalar_add(
                    out=qf[:, 0, :], in0=qk[:, 0, :], scalar1=bqkv[:, 0:1]
                )
                nc.vector.tensor_scalar_add(
                    out=qf[:, 1, :], in0=qk[:, 1, :], scalar1=bqkv[:, 1:2]
                )
                nc.sync.dma_start(out=qk8_d[:, :, i, :], in_=qf[:])
                for h in range(HPC):
                    for gg in range(2):
                        r0 = h * 64 + gg * 32
                        nc.sync.dma_start(
                            out=qk8[:, h, gg, :, i, :],
                            in_=qk8_d[r0 : r0 + 32, :, i, :],
                        )

            def qkv_v(i, half):
                if half == 0:
                    vv = ps.tile([128, 4, 128], f32, tag="sc", bufs=SC_BUFS, name="vv")
                    qkv_state[(i, "vv")] = vv
                    rr = range(2)
                else:
                    vv = qkv_state[(i, "vv")]
                    rr = range(2, 4)
                for r in rr:
                    jt = 4 * i + r
                    for ct in range(4):
                        nc.tensor.matmul(
                            vv[:, r, :],
                            lhsT=xt[ct][:, jt * JTW : (jt + 1) * JTW],
                            rhs=wqkv[:, ct, 256:384],
                            start=(ct == 0),
                            stop=(ct == 3),
                        )
                if half == 1:
                    del qkv_state[(i, "vv")]
                    for r in range(4):
                        jt = 4 * i + r
                        nc.vector.tensor_copy(
                            out=vno[:, jt, :].rearrange(
                                "p (h dho) -> p h dho", h=HPC
                            )[:, :, 0:Dh],
                            in_=vv[:, r, :].rearrange("p (h d) -> p h d", h=HPC),
                        )

            # ic-0 interleave schedule: unit -> qkv task
            unit_tasks = {}
            for c in range(3, NIC):
                unit_tasks[3 * (c - 3) + 0] = ("qk", c)
                unit_tasks[3 * (c - 3) + 2] = ("rp", c)
            for c in range(2, NIC):
                unit_tasks[15 + 2 * (c - 2) + 0] = ("vA", c)
                unit_tasks[15 + 2 * (c - 2) + 1] = ("vB", c)

            def run_task(task):
                kind, c = task
                if kind == "qk":
                    qkv_qk(c)
                elif kind == "rp":
                    qkv_repack(c)
                elif kind == "vA":
                    qkv_v(c, 0)
                else:
                    qkv_v(c, 1)

            def emit_scores(u):
                ic, jt = u
                sc = ps.tile([128, 2, ICW], f32, tag="sc", bufs=SC_BUFS, name="sc")
                for h in range(HPC):
                    nc.tensor.matmul(
                        sc[:, h, :],
                        lhsT=qk8[
                            :, h, :, 1, jt // 4,
                            (jt % 4) * JTW : (jt % 4 + 1) * JTW,
                        ],
                        rhs=qk8[:, h, :, 0, ic, :],
                        start=True,
                        stop=True,
                        perf_mode=DR,
                    )
                return sc

            def emit_proj(ic):
                isl = slice(ic * ICW, (ic + 1) * ICW)
                for cc in range(4):
                    pp = ps.tile([128, 2, ICW], f32, tag="sc", bufs=SC_BUFS, name="pp")
                    nc.tensor.matmul(
                        pp[:, 0, :],
                        lhsT=wp[:, cc * 128 : (cc + 1) * 128],
                        rhs=onorm[:, isl],
                        start=True,
                        stop=True,
                    )
                    st = sp.tile([128, ICW], f32, tag="st", bufs=2, name="st")
                    nc.vector.tensor_copy(out=st[:], in_=pp[:, 0, :])
                    nc.sync.dma_start(
                        out=out_d[cc * 128 : (cc + 1) * 128, isl], in_=st[:]
                    )

            # --- attention (software-pipelined over units u = (ic, jt)) ---
            for _rep in range(reps):
                load_x()
                # prologue: chunks 0,1 fully; chunk 2 q/k (+repack in flight)
                qkv_qk(0)
                qkv_repack(0)
                qkv_qk(1)
                qkv_repack(1)
                qkv_v(0, 0)
                qkv_v(0, 1)
                qkv_v(1, 0)
                qkv_v(1, 1)
                qkv_qk(2)
                qkv_repack(2)

                units = [(ic, jt) for ic in range(NIC) for jt in range(NJT)]
                accs = {}
                acc_emitted = {}
                p2s = {}
                pending_proj = None
                pending_attn = []

                sc_tiles = {0: emit_scores(units[0])}
                emitted = [0]

                def ensure_scores(upto):
                    while emitted[0] < min(upto, len(units) - 1):
                        emitted[0] += 1
                        sc_tiles[emitted[0]] = emit_scores(units[emitted[0]])

                def attn_pair(ic, t):
                    if ic not in accs:
                        accs[ic] = [
                            ps.tile([Dh + 1, ICW], f32, tag=f"acc{h}", bufs=1,
                                    name=f"acc{h}")
                            for h in range(HPC)
                        ]
                        acc_emitted[ic] = 0
                    p2 = p2s.pop((ic, t))
                    first = acc_emitted[ic] == 0
                    acc_emitted[ic] += 1
                    last = acc_emitted[ic] == NPAIR
                    for h in range(HPC):
                        nc.tensor.matmul(
                            accs[ic][h][0 : Dh + 1, :],
                            lhsT=vno[
                                :, 2 * t : 2 * t + 2,
                                h * (Dh + 1) : (h + 1) * (Dh + 1),
                            ],
                            rhs=p2[:, :, h, :],
                            start=first,
                            stop=last,
                            perf_mode=DR,
                        )

                for idx, (ic, jt) in enumerate(units):
                    isl = slice(ic * ICW, (ic + 1) * ICW)
                    t, g = divmod(jt, 2)
                    if g == 0:
                        p2s[(ic, t)] = sp.tile(
                            [128, 2, HPC, ICW], fp8, tag="p2", bufs=P2_BUFS, name="p2"
                        )
                    p2 = p2s[(ic, t)]
                    sc = sc_tiles.pop(idx)
                    if is_pool(idx):
                        stg = sp.tile(
                            [128, 2, ICW], bf16, tag="stg", bufs=STG_BUFS, name="stg"
                        )
                        nc.vector.tensor_copy(out=stg[:], in_=sc[:])
                        nc.gpsimd.tensor_tensor(
                            out=p2[:, g, :, :], in0=base[:], in1=stg[:], op=pow_op
                        )
                    else:
                        nc.scalar.activation(p2[:, g, :, :], sc[:], Exp, scale=SCALE)
                    # PE: upcoming scores (keeps exp engines fed)
                    ensure_scores(idx + (2 if jt in (NJT - 2, NJT - 1, 0) else 1))
                    # ic-0: interleaved qkv work for chunks 2..7
                    if ic == 0 and jt in unit_tasks:
                        run_task(unit_tasks[jt])
                    if pending_proj is not None and jt == 5:
                        emit_proj(pending_proj)
                        pending_proj = None
                    if g == 1:
                        pending_attn.append((ic, t))
                    # attnV emission policy
                    if ic == 0:
                        if jt >= FLUSH0:
                            n = max(1, (2 * NPAIR) // (NJT - FLUSH0 + 1))
                            for _ in range(min(n, len(pending_attn))):
                                attn_pair(*pending_attn.pop(0))
                            if jt == NJT - 1:
                                while pending_attn:
                                    attn_pair(*pending_attn.pop(0))
                    else:
                        while len(pending_attn) > PDEFER:
                            attn_pair(*pending_attn.pop(0))
                        if jt == NJT - 1:
                            while pending_attn:
                                attn_pair(*pending_attn.pop(0))
                    if jt != NJT - 1:
                        continue
                    # --- end of i-chunk: normalize O_raw by the denom row ---
                    for h in range(HPC):
                        acc = accs[ic][h]
                        if AB_DIRECT:
                            src = acc
                        else:
                            src = sp.tile(
                                [Dh + 1, ICW], f32, tag=f"ab{h}", bufs=2, name="ab"
                            )
                            nc.vector.tensor_copy(out=src[:], in_=acc[:])
                        rc = sp.tile([1, ICW], f32, tag=f"rc{h}", bufs=2, name="rc")
                        nc.vector.reciprocal(rc[:], src[Dh : Dh + 1, :])
                        rb = sp.tile([Dh, ICW], f32, tag=f"rb{h}", bufs=2, name="rb")
                        nc.gpsimd.partition_broadcast(rb[:], rc[:])
                        nc.vector.tensor_tensor(
                            out=onorm[h * Dh : (h + 1) * Dh, isl],
                            in0=src[0:Dh, :],
                            in1=rb[:],
                            op=mult,
                        )
                    pending_proj = ic
                if pending_proj is not None:
                    emit_proj(pending_proj)

    nc.compile()
    return nc


def get_nc(reps=1):
    if reps not in _cached_nc:
        _cached_nc[reps] = _build_nc(reps)
    return _cached_nc[reps]


def make_in_maps(x, qkv_w, qkv_b, proj_w):
    """Build the per-core input dicts (host-side sharding + layout prep)."""
    x = np.asarray(x, dtype=np.float32)
    qkv_w = np.asarray(qkv_w, dtype=np.float32)
    qkv_b = np.asarray(qkv_b, dtype=np.float32)
    proj_w = np.asarray(proj_w, dtype=np.float32)

    in_maps = []
    for c in range(NCORES):
        b, j = divmod(c, 4)
        rq = slice(128 * j, 128 * (j + 1))
        rk = slice(512 + 128 * j, 512 + 128 * (j + 1))
        rv = slice(1024 + 128 * j, 1024 + 128 * (j + 1))
        xt = np.ascontiguousarray(x[b].T).astype(_BF16)
        wqkv = np.ascontiguousarray(
            np.concatenate([qkv_w[rq].T, qkv_w[rk].T, qkv_w[rv].T], axis=1)
        ).astype(_BF16)
        wp = np.ascontiguousarray(proj_w[:, rq].T).astype(_BF16)
        bqkv = np.ascontiguousarray(
            np.stack([qkv_b[rq], qkv_b[rk]], axis=1)
        ).astype(np.float32)
        in_maps.append({"xt": xt, "wqkv": wqkv, "wp": wp, "bqkv": bqkv})
    return in_maps


def gather_output(results, proj_b, qkv_b, proj_w):
    """Sum per-core projection partials per batch, transpose, add bias.

    The v-bias is folded in here: out += proj_w @ vb + proj_b.
    """
    proj_b = np.asarray(proj_b, dtype=np.float32)
    vb = np.asarray(qkv_b, dtype=np.float32)[2 * C : 3 * C]
    pb_eff = proj_b + np.asarray(proj_w, dtype=np.float32) @ vb
    out = np.empty((B, N, C), dtype=np.float32)
    for b in range(B):
        acc = np.zeros((C, N), dtype=np.float32)
        for j in range(4):
            acc += np.asarray(results[4 * b + j]["out"], dtype=np.float32)
        out[b] = acc.T + pb_eff
    return out


def kernel(x, qkv_w, qkv_b, proj_w, proj_b):
    from concourse.bass_utils import run_bass_kernel_spmd

    nc = get_nc()
    in_maps = make_in_maps(x, qkv_w, qkv_b, proj_w)
    res = run_bass_kernel_spmd(nc, in_maps, list(range(NCORES)))
    return gather_output(res.results, proj_b, qkv_b, proj_w)


def run_traced(x, qkv_w, qkv_b, proj_w, proj_b, trace_cores=None):
    """Like kernel(), but profiles and returns (out, exec_time_ns, raw result)."""
    from concourse.bass_utils import run_bass_kernel_spmd

    nc = get_nc()
    in_maps = make_in_maps(x, qkv_w, qkv_b, proj_w)
    res = run_bass_kernel_spmd(
        nc, in_maps, list(range(NCORES)), trace=True, trace_cores=trace_cores
    )
    return gather_output(res.results, proj_b, qkv_b, proj_w), res.exec_time_ns, res


# revision 11
# speedup vs baseline: 1.0951x; 1.0951x over previous
"""Multi-head attention (B=2, N=4096, C=512, H=8) on 8 TRN2 NeuronCores.

Sharding: core c handles batch b = c//4 and heads {2*(c%4), 2*(c%4)+1}
(data parallel over B, tensor parallel over heads).  Each core computes its
two heads' full attention plus their slice of the output projection; the
per-core [C, N] projection partials are summed per batch on the host
(the "proj all-reduce") and the projection bias is added there too (with
the attention v-bias folded in: out = Wp(O0 + vb) + pb = Wp O0 + (Wp vb + pb)).

Device-side math per core:
  qT/kT   = Wq/Wk_blk @ x^T + b           [128 (2h x 64d), 4096] fp32 PSUM
  q8/k8   = fp8e4(qT/kT)                  repacked via DRAM roundtrip into
                                          [32 (d0), 2h, 2g, 2t, 8ic, 512] so the
                                          score matmul runs fp8 DoubleRow with
                                          the 64-d contraction split into 2
                                          groups of 32 on partitions 0:32.
  v_nat   = x_jt^T @ Wv  (vb folded out)  [128 (j), 2h x 64d] -> vno fp8
                                          [128, 32jt, (64|1|64|1)] ones cols
  S^T     = DR-matmul(k8, q8)             [128 (j), 2h, 512i] fp32 PSUM
  P^T     = exp(SCALE * S^T) -> fp8       ACT exp (most units) or DVE
                                          bf16-stage + GPSIMD pow (rest)
  O_raw^T = DR-matmul over jt PAIRS:      lhsT = vno[:, 2t:2t+2, h slice],
            rhs = p2[:, 2g, h, :]         -> acc[h] [65, 512] (row 64 = denom)
  O^T     = O_raw^T * (1/denom)           DVE recip + Pool bcast + DVE mult
  out^T  += Wp_blk^T @ O^T                [512, 4096] fp32 -> DRAM
"""

import os
import numpy as np
import ml_dtypes

SC_BUFS = int(os.environ.get("SC_BUFS", "3"))
P2_BUFS = int(os.environ.get("P2_BUFS", "18"))
STG_BUFS = int(os.environ.get("STG_BUFS", "4"))
PDEFER = int(os.environ.get("PDEFER", "4"))
POOL_NUM = int(os.environ.get("POOL_NUM", "7"))   # pool pairs per POOL_DEN
POOL_DEN = int(os.environ.get("POOL_DEN", "10"))
POOL_NUM0 = int(os.environ.get("POOL_NUM0", "1"))  # ...during i-chunk 0
POOL_DEN0 = int(os.environ.get("POOL_DEN0", "2"))
AB_DIRECT = int(os.environ.get("AB_DIRECT", "1"))
WARM = int(os.environ.get("WARM", "24"))
FLUSH0 = int(os.environ.get("FLUSH0", "27"))  # unit where ic-0 attnV flush starts

B, N, C = 2, 4096, 512
H, Dh = 8, 64
SCALE = Dh**-0.5
NCORES = 8
HPC = 2  # heads per core
ICW = 512  # i-chunk width
NIC = N // ICW  # 8
JTW = 128  # j-tile width
NJT = N // JTW  # 32
NPAIR = NJT // 2  # 16 jt pairs per i-chunk

_BF16 = ml_dtypes.bfloat16
_F8 = ml_dtypes.float8_e4m3

_cached_nc = {}


def _build_nc(reps=1):
    import concourse.bacc as bacc
    import concourse.tile as tile
    import concourse.mybir as mybir

    f32 = mybir.dt.float32
    bf16 = mybir.dt.bfloat16
    fp8 = mybir.dt.float8e4
    Exp = mybir.ActivationFunctionType.Exp
    mult = mybir.AluOpType.mult
    pow_op = mybir.AluOpType.pow
    DR = mybir.MatmulPerfMode.DoubleRow

    nc = bacc.Bacc("TRN2", target_bir_lowering=False, debug=False)

    xt_d = nc.dram_tensor("xt", [C, N], bf16, kind="ExternalInput").ap()
    wqkv_d = nc.dram_tensor("wqkv", [C, 3 * 128], bf16, kind="ExternalInput").ap()
    wp_d = nc.dram_tensor("wp", [128, C], bf16, kind="ExternalInput").ap()
    bqkv_d = nc.dram_tensor("bqkv", [128, 2], f32, kind="ExternalInput").ap()
    out_d = nc.dram_tensor("out", [C, N], f32, kind="ExternalOutput").ap()
    # DRAM scratch for the q8/k8 partition repack: [p, t(q|k), ic, i]
    qk8_d = nc.dram_tensor("qk8s", [128, 2, NIC, ICW], fp8, kind="ExternalOutput").ap()

    def is_pool(idx):
        # Pair-aware: at most one Pool unit per jt-pair (so ACT+Pool overlap
        # within a pair), alternating which member, Bresenham-spread pairs.
        t, g = divmod(idx, 2)
        if idx < NJT:
            pair_pool = (t * POOL_NUM0) % POOL_DEN0 < POOL_NUM0
        else:
            pair_pool = (t * POOL_NUM) % POOL_DEN < POOL_NUM
        return pair_pool and g == (t % 2)

    with tile.TileContext(nc) as tc:
        with (
            tc.tile_pool(name="ps", space="PSUM", bufs=2) as ps,
            tc.tile_pool(name="sp", bufs=2) as sp,
            tc.tile_pool(name="pe", bufs=1) as pe,
        ):
            # --- persistent SBUF tensors ---
            xt = [
                pe.tile([128, N], bf16, tag=f"xt{i}", name=f"xt{i}") for i in range(4)
            ]

            def load_x():
                # 8 large DMAs (HWDGE fixed cost dominates small ones): the
                # first 4 cover columns 0:2048 of every ct tile (enough for
                # qkv chunks 0-3), the rest follow.
                for half in range(2):
                    for ct in range(4):
                        cs = slice(half * (N // 2), (half + 1) * (N // 2))
                        nc.sync.dma_start(
                            out=xt[ct][:, cs],
                            in_=xt_d[ct * 128 : (ct + 1) * 128, cs],
                        )

            wqkv = pe.tile([128, 4, 3 * 128], bf16, tag="wqkv", name="wqkv")
            nc.sync.dma_start(
                out=wqkv[:], in_=wqkv_d.rearrange("(ct p) m -> p ct m", p=128)
            )
            wp = pe.tile([128, C], bf16, tag="wp", name="wp")
            nc.sync.dma_start(out=wp[:], in_=wp_d[:, :])
            bqkv = pe.tile([128, 2], f32, tag="bqkv", name="bqkv")
            nc.sync.dma_start(out=bqkv[:], in_=bqkv_d[:, :])

            # q8/k8 in d-split layout: [32 (d0), h, g, t, ic, i]
            qk8 = pe.tile([32, HPC, 2, 2, NIC, ICW], fp8, tag="qk8", name="qk8")
            # v natural layout + ones cols: [j0, jt, (64 v | 1 | 64 v | 1)]
            vno = pe.tile([128, NJT, HPC * (Dh + 1)], fp8, tag="vno", name="vno")
            onorm = pe.tile([128, N], bf16, tag="onorm", name="onorm")
            base = pe.tile([128, HPC, ICW], f32, tag="base", name="base")
            nc.vector.memset(base[:], float(np.exp(SCALE)))
            nc.vector.memset(vno[:, :, Dh : Dh + 1], 1.0)
            nc.vector.memset(vno[:, :, 2 * Dh + 1 : 2 * Dh + 2], 1.0)

            # PE pre-warm (pstate ramp) while input DMAs land.
            warm = pe.tile([128, 128], bf16, tag="warm", name="warm")
            nc.vector.memset(warm[:], 0.0)
            wps = ps.tile([128, HPC, ICW], f32, tag="sc", bufs=SC_BUFS, name="wps")
            for _ in range(WARM):
                nc.tensor.matmul(
                    wps[:, 0, 0:128], lhsT=warm[:], rhs=warm[:], start=True, stop=True
                )

            # --- QKV production for one 512-column chunk, in parts ---
            qkv_state = {}

            def qkv_qk(i):
                isl = slice(i * ICW, (i + 1) * ICW)
                qk = ps.tile([128, 2, ICW], f32, tag="sc", bufs=SC_BUFS, name="qk")
                qkv_state[i] = qk
                for part in range(2):
                    for ct in range(4):
                        nc.tensor.matmul(
                            qk[:, part, :],
                            lhsT=wqkv[:, ct, part * 128 : (part + 1) * 128],
                            rhs=xt[ct][:, isl],
                            start=(ct == 0),
                            stop=(ct == 3),
                        )

            def qkv_repack(i):
                qk = qkv_state.pop(i)
                qf = sp.tile([128, 2, ICW], fp8, tag="qf", bufs=2, name="qf")
                nc.vector.tensor_scalar_add(
                    out=qf[:, 0, :], in0=qk[:, 0, :], scalar1=bqkv[:, 0:1]
                )
                nc.vector.tensor_scalar_add(
                    out=qf[:, 1, :], in0=qk[:, 1, :], scalar1=bqkv[:, 1:2]
                )
                nc.sync.dma_start(out=qk8_d[:, :, i, :], in_=qf[:])
                for h in range(HPC):
                    for gg in range(2):
                        r0 = h * 64 + gg * 32
                        nc.sync.dma_start(
                            out=qk8[:, h, gg, :, i, :],
                            in_=qk8_d[r0 : r0 + 32, :, i, :],
                        )

            def qkv_v(i, half):
                if half == 0:
                    vv = ps.tile([128, 4, 128], f32, tag="sc", bufs=SC_BUFS, name="vv")
                    qkv_state[(i, "vv")] = vv
                    rr = range(2)
                else:
                    vv = qkv_state[(i, "vv")]
                    rr = range(2, 4)
                for r in rr:
                    jt = 4 * i + r
                    for ct in range(4):
                        nc.tensor.matmul(
                            vv[:, r, :],
                            lhsT=xt[ct][:, jt * JTW : (jt + 1) * JTW],
                            rhs=wqkv[:, ct, 256:384],
                            start=(ct == 0),
                            stop=(ct == 3),
                        )
                if half == 1:
                    del qkv_state[(i, "vv")]
                    for r in range(4):
                        jt = 4 * i + r
                        nc.vector.tensor_copy(
                            out=vno[:, jt, :].rearrange(
                                "p (h dho) -> p h dho", h=HPC
                            )[:, :, 0:Dh],
                            in_=vv[:, r, :].rearrange("p (h d) -> p h d", h=HPC),
                        )

            # ic-0 interleave schedule: unit -> qkv task
            unit_tasks = {}
            for c in range(3, NIC):
                unit_tasks[3 * (c - 3) + 0] = ("qk", c)
                unit_tasks[3 * (c - 3) + 2] = ("rp", c)
            for c in range(2, NIC):
                unit_tasks[15 + 2 * (c - 2) + 0] = ("vA", c)
                unit_tasks[15 + 2 * (c - 2) + 1] = ("vB", c)

            def run_task(task):
                kind, c = task
                if kind == "qk":
                    qkv_qk(c)
                elif kind == "rp":
                    qkv_repack(c)
                elif kind == "vA":
                    qkv_v(c, 0)
                else:
                    qkv_v(c, 1)

            def emit_scores(u):
                ic, jt = u
                sc = ps.tile([128, 2, ICW], f32, tag="sc", bufs=SC_BUFS, name="sc")
                for h in range(HPC):
                    nc.tensor.matmul(
                        sc[:, h, :],
                        lhsT=qk8[
                            :, h, :, 1, jt // 4,
                            (jt % 4) * JTW : (jt % 4 + 1) * JTW,
                        ],
                        rhs=qk8[:, h, :, 0, ic, :],
                        start=True,
                        stop=True,
                        perf_mode=DR,
                    )
                return sc

            def emit_proj_part(ic, cc):
                isl = slice(ic * ICW, (ic + 1) * ICW)
                pp = ps.tile([128, 2, ICW], f32, tag="sc", bufs=SC_BUFS, name="pp")
                nc.tensor.matmul(
                    pp[:, 0, :],
                    lhsT=wp[:, cc * 128 : (cc + 1) * 128],
                    rhs=onorm[:, isl],
                    start=True,
                    stop=True,
                )
                st = sp.tile([128, ICW], f32, tag="st", bufs=2, name="st")
                nc.vector.tensor_copy(out=st[:], in_=pp[:, 0, :])
                nc.sync.dma_start(
                    out=out_d[cc * 128 : (cc + 1) * 128, isl], in_=st[:]
                )

            # --- attention (software-pipelined over units u = (ic, jt)) ---
            for _rep in range(reps):
                load_x()
                # prologue: chunks 0,1 fully; chunk 2 q/k (+repack in flight)
                qkv_qk(0)
                qkv_repack(0)
                qkv_qk(1)
                qkv_repack(1)
                qkv_v(0, 0)
                qkv_v(0, 1)
                qkv_v(1, 0)
                qkv_v(1, 1)
                qkv_qk(2)
                qkv_repack(2)

                units = [(ic, jt) for ic in range(NIC) for jt in range(NJT)]
                accs = {}
                acc_emitted = {}
                p2s = {}
                pending_proj = None
                pending_attn = []

                sc_tiles = {0: emit_scores(units[0])}
                emitted = [0]

                def ensure_scores(upto):
                    while emitted[0] < min(upto, len(units) - 1):
                        emitted[0] += 1
                        sc_tiles[emitted[0]] = emit_scores(units[emitted[0]])

                def attn_pair(ic, t):
                    if ic not in accs:
                        accs[ic] = [
                            ps.tile([Dh + 1, ICW], f32, tag=f"acc{h}", bufs=1,
                                    name=f"acc{h}")
                            for h in range(HPC)
                        ]
                        acc_emitted[ic] = 0
                    p2 = p2s.pop((ic, t))
                    first = acc_emitted[ic] == 0
                    acc_emitted[ic] += 1
                    last = acc_emitted[ic] == NPAIR
                    for h in range(HPC):
                        nc.tensor.matmul(
                            accs[ic][h][0 : Dh + 1, :],
                            lhsT=vno[
                                :, 2 * t : 2 * t + 2,
                                h * (Dh + 1) : (h + 1) * (Dh + 1),
                            ],
                            rhs=p2[:, :, h, :],
                            start=first,
                            stop=last,
                            perf_mode=DR,
                        )

                def normalize(ic, h):
                    isl = slice(ic * ICW, (ic + 1) * ICW)
                    acc = accs[ic][h]
                    if AB_DIRECT:
                        src = acc
                    else:
                        src = sp.tile(
                            [Dh + 1, ICW], f32, tag=f"ab{h}", bufs=2, name="ab"
                        )
                        nc.vector.tensor_copy(out=src[:], in_=acc[:])
                    rc = sp.tile([1, ICW], f32, tag=f"rc{h}", bufs=2, name="rc")
                    nc.vector.reciprocal(rc[:], src[Dh : Dh + 1, :])
                    rb = sp.tile([Dh, ICW], f32, tag=f"rb{h}", bufs=2, name="rb")
                    nc.gpsimd.partition_broadcast(rb[:], rc[:])
                    nc.vector.tensor_tensor(
                        out=onorm[h * Dh : (h + 1) * Dh, isl],
                        in0=src[0:Dh, :],
                        in1=rb[:],
                        op=mult,
                    )

                pending_norm = None
                for idx, (ic, jt) in enumerate(units):
                    t, g = divmod(jt, 2)
                    if g == 0:
                        p2s[(ic, t)] = sp.tile(
                            [128, 2, HPC, ICW], fp8, tag="p2", bufs=P2_BUFS, name="p2"
                        )
                    p2 = p2s[(ic, t)]
                    sc = sc_tiles.pop(idx)
                    if is_pool(idx):
                        stg = sp.tile(
                            [128, 2, ICW], bf16, tag="stg", bufs=STG_BUFS, name="stg"
                        )
                        nc.vector.tensor_copy(out=stg[:], in_=sc[:])
                        nc.gpsimd.tensor_tensor(
                            out=p2[:, g, :, :], in0=base[:], in1=stg[:], op=pow_op
                        )
                    else:
                        nc.scalar.activation(p2[:, g, :, :], sc[:], Exp, scale=SCALE)
                    # PE: upcoming scores (keeps exp engines fed)
                    ensure_scores(idx + (2 if jt in (NJT - 2, NJT - 1, 0, 1) else 1))
                    # ic-0: interleaved qkv work for chunks 2..7
                    if ic == 0 and jt in unit_tasks:
                        run_task(unit_tasks[jt])
                    # deferred second-head normalize from the previous chunk
                    if pending_norm is not None and jt == 2:
                        normalize(*pending_norm)
                        pending_norm = None
                    # proj of the previous chunk, one cc part per even unit
                    if pending_proj is not None and jt in (8, 10, 12, 14):
                        emit_proj_part(pending_proj, (jt - 8) // 2)
                        if jt == 14:
                            pending_proj = None
                    if g == 1:
                        pending_attn.append((ic, t))
                    # attnV emission policy
                    if ic == 0:
                        if jt >= FLUSH0:
                            n = max(1, (2 * NPAIR) // (NJT - FLUSH0 + 1))
                            for _ in range(min(n, len(pending_attn))):
                                attn_pair(*pending_attn.pop(0))
                            if jt == NJT - 1:
                                while pending_attn:
                                    attn_pair(*pending_attn.pop(0))
                    else:
                        while len(pending_attn) > PDEFER:
                            attn_pair(*pending_attn.pop(0))
                        if jt == NJT - 1:
                            while pending_attn:
                                attn_pair(*pending_attn.pop(0))
                    if jt != NJT - 1:
                        continue
                    # --- end of i-chunk: normalize (h0 now, h1 two units in) ---
                    normalize(ic, 0)
                    if ic < NIC - 1:
                        pending_norm = (ic, 1)
                    else:
                        normalize(ic, 1)
                    pending_proj = ic
                if pending_norm is not None:
                    normalize(*pending_norm)
                for cc in range(4):
                    emit_proj_part(pending_proj, cc)

    nc.compile()
    return nc


def get_nc(reps=1):
    if reps not in _cached_nc:
        _cached_nc[reps] = _build_nc(reps)
    return _cached_nc[reps]


def make_in_maps(x, qkv_w, qkv_b, proj_w):
    """Build the per-core input dicts (host-side sharding + layout prep)."""
    x = np.asarray(x, dtype=np.float32)
    qkv_w = np.asarray(qkv_w, dtype=np.float32)
    qkv_b = np.asarray(qkv_b, dtype=np.float32)
    proj_w = np.asarray(proj_w, dtype=np.float32)

    in_maps = []
    for c in range(NCORES):
        b, j = divmod(c, 4)
        rq = slice(128 * j, 128 * (j + 1))
        rk = slice(512 + 128 * j, 512 + 128 * (j + 1))
        rv = slice(1024 + 128 * j, 1024 + 128 * (j + 1))
        xt = np.ascontiguousarray(x[b].T).astype(_BF16)
        wqkv = np.ascontiguousarray(
            np.concatenate([qkv_w[rq].T, qkv_w[rk].T, qkv_w[rv].T], axis=1)
        ).astype(_BF16)
        wp = np.ascontiguousarray(proj_w[:, rq].T).astype(_BF16)
        bqkv = np.ascontiguousarray(
            np.stack([qkv_b[rq], qkv_b[rk]], axis=1)
        ).astype(np.float32)
        in_maps.append({"xt": xt, "wqkv": wqkv, "wp": wp, "bqkv": bqkv})
    return in_maps


def gather_output(results, proj_b, qkv_b, proj_w):
    """Sum per-core projection partials per batch, transpose, add bias.

    The v-bias is folded in here: out += proj_w @ vb + proj_b.
    """
    proj_b = np.asarray(proj_b, dtype=np.float32)
    vb = np.asarray(qkv_b, dtype=np.float32)[2 * C : 3 * C]
    pb_eff = proj_b + np.asarray(proj_w, dtype=np.float32) @ vb
    out = np.empty((B, N, C), dtype=np.float32)
    for b in range(B):
        acc = np.zeros((C, N), dtype=np.float32)
        for j in range(4):
            acc += np.asarray(results[4 * b + j]["out"], dtype=np.float32)
        out[b] = acc.T + pb_eff
    return out


def kernel(x, qkv_w, qkv_b, proj_w, proj_b):
    from concourse.bass_utils import run_bass_kernel_spmd

    nc = get_nc()
    in_maps = make_in_maps(x, qkv_w, qkv_b, proj_w)
    res = run_bass_kernel_spmd(nc, in_maps, list(range(NCORES)))
    return gather_output(res.results, proj_b, qkv_b, proj_w)


def run_traced(x, qkv_w, qkv_b, proj_w, proj_b, trace_cores=None):
    """Like kernel(), but profiles and returns (out, exec_time_ns, raw result)."""
    from concourse.bass_utils import run_bass_kernel_spmd

    nc = get_nc()
    in_maps = make_in_maps(x, qkv_w, qkv_b, proj_w)
    res = run_bass_kernel_spmd(
        nc, in_maps, list(range(NCORES)), trace=True, trace_cores=trace_cores
    )
    return gather_output(res.results, proj_b, qkv_b, proj_w), res.exec_time_ns, res
